# revision 9
# baseline (speedup 1.0000x reference)
"""Depth-aware forward-warp (bilinear splat) + flow add, on 8 trn2 cores.

Strategy: dense shifted-window splat, y-on-partitions layout.

Each image is cut into 5 row-strips of 128 source rows (stride 116, so each
strip's middle 116 rows receive complete contributions; +-6 halo covers
max |displacement| ~5.5 for N(0,1) data). Within a strip tile [128p, 960f]:
  partitions = source rows, free dim = x columns.

For integer offsets (dy, dx), the contribution of source pixel (r, x) to
dest (r+dy, x+dx) is v * tri(bx - dx) * tri(by - dy), tri(u) = relu(1-|u|).
  - dx shifts are free-dim AP offsets (clipped at image edge -> exact
    boundary handling, no halo needed in x).
  - dy shifts + accumulation happen inside the PE matmul: stationary is a
    0/1 band (shift) matrix S_dy with S[q, p] = 1 iff p == q+dy, so
    psum[p, f+dx] += sum_q S[q,p] * tm[q, f] scatters rows while PSUM
    accumulates. No shift DMAs at all.
Everything is fp16 (DVE 2x, PE 1 cyc/row) except PSUM/normalization (fp32).

Per (dy, dx) combo: one packed DVE mul tm[c] = h[dx][c] * ty[dy] (c = 3
channels: flowBC.x, flowBC.y, ones, all pre-scaled by exp(-depth)), then
6 matmuls (3 channels x 2 psum banks). h and ty planes are hoisted:
h[dx] = v * tri(bx-dx) once per dx, ty[dy] = tri(by-dy) once per dy.
"""
import sys
sys.path.insert(0, '/opt/trn_rl_repo')
import numpy as np
import concourse.bacc as bacc
import concourse.mybir as mybir
from concourse.tile import TileContext
from concourse.bass_utils import run_bass_kernel_spmd

AF = mybir.ActivationFunctionType
ALU = mybir.AluOpType
F16 = mybir.dt.float16
F32 = mybir.dt.float32

# Weight-plane scaling: tri factors are stored as 64*tri in fp16 and the
# ones-channel carries e*exp(-dep) = exp(1-dep), so every intermediate that
# can decide the den>eps mask stays in fp16 *normal* range (no reliance on
# subnormals). The combined scale e*64*64 cancels in the num/den ratio; only
# the eps threshold is rescaled.
SW = 64.0
THRESH = float(np.exp(1.0) * SW * SW * 1e-6)

B, H, W = 16, 540, 960
NCORES = 8
IPC = B // NCORES          # images per core (2)
NS = 5                     # strips per image
STRIDE = 116               # complete dest rows per strip
HALO = 6                   # max |dy|
SLOTS = IPC * NS           # chunks per core (10)
PP = 128
BANDW = PP + 2 * HALO      # 140
PADV = np.float32(1000.0)
EPS = np.float32(1e-6)

_CACHE = {}


def _strip_rows(si):
    """Source-row window [y0, y0+128) of strip si; dest rows [y0+6, ...)."""
    y0 = -HALO + STRIDE * si
    return y0


def _active_sets(bx16, by16):
    """Per slot: ordered ((dy, ((dx, a, b), ...)), ...) unioned across cores.

    a, b = source-column range (already clipped for x-image-boundary
    validity). bx16/by16: [B, H, W] fp16-rounded displacements (fp32 vals).
    """
    out = []
    xs_row = np.arange(W, dtype=np.int32)
    for ki in range(IPC):
        for si in range(NS):
            y0 = _strip_rows(si)
            lo, hi = max(0, y0), min(H, y0 + PP)
            combos = {}
            for core in range(NCORES):
                b = core * IPC + ki
                fx = np.floor(bx16[b, lo:hi]).astype(np.int64)
                fy = np.floor(by16[b, lo:hi]).astype(np.int64)
                xs = np.broadcast_to(xs_row[None, :], fx.shape).ravel()
                for ox in (0, 1):
                    for oy in (0, 1):
                        key = (fy.ravel() + oy) * 4096 + (fx.ravel() + ox)
                        order = np.argsort(key, kind='stable')
                        ks, xo = key[order], xs[order]
                        uniq, idx = np.unique(ks, return_index=True)
                        mins = np.minimum.reduceat(xo, idx)
                        maxs = np.maximum.reduceat(xo, idx)
                        for u, mn, mx in zip(uniq, mins, maxs):
                            dy, dx = int(u) // 4096, int(u) % 4096
                            if dx > 2048:
                                dx -= 4096
                                dy += 1
                            if abs(dy) > HALO:
                                raise ValueError(f"dy={dy} out of halo")
                            c = combos.get((dy, dx))
                            if c is None:
                                combos[(dy, dx)] = [int(mn), int(mx)]
                            else:
                                c[0] = min(c[0], int(mn))
                                c[1] = max(c[1], int(mx))
            per_dy = {}
            for (dy, dx), (mn, mx) in sorted(combos.items()):
                a = max(mn, 0, -dx)
                bb = min(mx + 1, W, W - dx)
                if bb - a <= 0:
                    continue
                per_dy.setdefault(dy, []).append((dx, a, bb))
            out.append(tuple(
                (dy, tuple(per_dy[dy])) for dy in sorted(per_dy)))
    return tuple(out)


def _register_consts(nc):
    vals = {0.0, 1.0, SW}
    for d in range(-HALO - 1, HALO + 2):
        vals.add(float(-d))
    for v in vals:
        key = (F32, float(v))
        if key in nc.const_aps.aps:
            continue
        t = nc.alloc_sbuf_tensor(f"constf32_{v}", [PP, 1], F32)
        nc.gpsimd.memset(t.ap(), float(v))
        nc.const_aps.aps[key] = t.ap()


def build_program(active, reps=1, n_cores=NCORES):
    nc = bacc.Bacc(trn_type="TRN2", debug=False, num_devices=n_cores)
    _register_consts(nc)

    def param(name, out=False, dt=F16):
        return nc.declare_dram_parameter(name, [SLOTS, PP, W], dt, isOutput=out)

    bx_d, by_d = param("bx", dt=F32), param("by", dt=F32)
    fbx_d, fby_d, dep_d = param("fbx"), param("fby"), param("dep")
    fax_d, fay_d = param("fax"), param("fay")
    outx_d, outy_d = param("outx", True), param("outy", True)
    band_d = nc.declare_dram_parameter("band", [PP, BANDW], F16, isOutput=False)

    with TileContext(nc) as tc:
        from contextlib import ExitStack
        with ExitStack() as ctx:
            c_pool = ctx.enter_context(tc.tile_pool(name="c", bufs=1))
            band = c_pool.tile([PP, BANDW], F16, tag="band", name="band_t")
            nc.sync.dma_start(out=band[:], in_=band_d[:])

            io_pool = ctx.enter_context(tc.tile_pool(name="io", bufs=2))
            v_pool = ctx.enter_context(tc.tile_pool(name="v", bufs=2))
            ty_pool = ctx.enter_context(tc.tile_pool(name="ty", bufs=1))
            h_pool = ctx.enter_context(tc.tile_pool(name="h", bufs=1))
            w_pool = ctx.enter_context(tc.tile_pool(name="w", bufs=2))
            tm_pool = ctx.enter_context(tc.tile_pool(name="tm", bufs=3))
            d_pool = ctx.enter_context(tc.tile_pool(name="d", bufs=1))
            o_pool = ctx.enter_context(tc.tile_pool(name="o", bufs=2))
            psum_pool = ctx.enter_context(
                tc.tile_pool(name="psum", bufs=1, space="PSUM"))

            def slot_body(k):
                act = active[k]
                dys = [dy for dy, _ in act]
                dxs = sorted({dx for _, l in act for dx, _, _ in l})
                ncomb = sum(len(l) for _, l in act)

                # --- inputs ---
                SH = [PP, W]
                bx = io_pool.tile(SH, F32, tag="bx", name="bx_t")
                by = io_pool.tile(SH, F32, tag="by", name="by_t")
                fbx = io_pool.tile(SH, F16, tag="fbx", name="fbx_t")
                fby = io_pool.tile(SH, F16, tag="fby", name="fby_t")
                dep = io_pool.tile(SH, F16, tag="dep", name="dep_t")
                for t, d in ((bx, bx_d), (by, by_d), (fbx, fbx_d),
                             (fby, fby_d), (dep, dep_d)):
                    nc.sync.dma_start(out=t[:], in_=d[k])

                # --- v channels: v2 = exp(1-dep); v0 = fbx*v2; v1 = fby*v2 ---
                v = v_pool.tile([PP, 3, W], F16, tag="v", name="v_t")
                nc.scalar.activation(v[:, 2, :], dep[:], AF.Exp,
                                     bias=1.0, scale=-1.0)
                nc.vector.tensor_mul(v[:, 0, :], fbx[:], v[:, 2, :])
                nc.vector.tensor_mul(v[:, 1, :], fby[:], v[:, 2, :])

                # --- ty planes (one per dy): 64*tri(by - dy) ---
                NDY = len(dys)
                ty = ty_pool.tile([PP, NDY, W], F16, tag="ty", name="ty_t")
                for j, dy in enumerate(dys):
                    u = w_pool.tile(SH, F32, tag="uy", name="uy_t")
                    nc.scalar.activation(u[:], by[:], AF.Abs,
                                         bias=float(-dy), scale=1.0)
                    nc.scalar.activation(ty[:, j, :], u[:], AF.Relu,
                                         bias=SW, scale=-SW)

                # --- h planes (one per dx): h = v * 64*tri(bx - dx) ---
                hs = {}
                for dx in dxs:
                    u2 = w_pool.tile(SH, F32, tag="ux", name="ux_t")
                    tx = w_pool.tile(SH, F16, tag="tx", name="tx_t")
                    nc.scalar.activation(u2[:], bx[:], AF.Abs,
                                         bias=float(-dx), scale=1.0)
                    nc.scalar.activation(tx[:], u2[:], AF.Relu,
                                         bias=SW, scale=-SW)
                    h = h_pool.tile([PP, 3, W], F16, tag=f"h{dx}",
                                    name=f"h{dx}_t")
                    txb = tx[:].rearrange("p (c f) -> p c f", c=1) \
                               .broadcast_to([PP, 3, W])
                    nc.vector.tensor_mul(h[:], v[:], txb)
                    hs[dx] = h

                # --- psum accumulation over combos ---
                PW = 1024  # 2 banks per channel
                ps = [psum_pool.tile([PP, PW], F32, tag=f"ps{c}",
                                     name=f"ps{c}_t") for c in range(3)]
                # first/last combo index touching each bank
                first = [None, None]
                last = [None, None]
                ci = 0
                for dy, lst in act:
                    for dx, a, bb in lst:
                        da, db = a + dx, bb + dx
                        if da < 512:
                            if first[0] is None:
                                first[0] = ci
                            last[0] = ci
                        if db > 512:
                            if first[1] is None:
                                first[1] = ci
                            last[1] = ci
                        ci += 1
                assert first[0] is not None and first[1] is not None

                ci = 0
                for dy, lst in act:
                    off = HALO - dy
                    sta = band[:, off:off + PP]
                    jdy = dys.index(dy)
                    for dx, a, bb in lst:
                        n = bb - a
                        tm = tm_pool.tile([PP, 3, W], F16, tag="tm",
                                          name="tm_t")
                        tyb = ty[:, jdy, a:bb] \
                            .rearrange("p (c f) -> p c f", c=1) \
                            .broadcast_to([PP, 3, n])
                        nc.vector.tensor_mul(tm[:, :, a:bb],
                                             hs[dx][:, :, a:bb], tyb)
                        da, db = a + dx, bb + dx
                        segs = []
                        if da < 512:
                            segs.append((0, da, min(db, 512)))
                        if db > 512:
                            segs.append((1, max(da, 512), db))
                        for c in range(3):
                            for bank, s0, s1 in segs:
                                nc.tensor.matmul(
                                    ps[c][:, s0:s1],
                                    sta,
                                    tm[:, c, s0 - dx:s1 - dx],
                                    start=(ci == first[bank]),
                                    stop=(ci == last[bank]),
                                    skip_group_check=True,
                                )
                        ci += 1

                # --- drain + normalize + add flowAB ---
                fax = io_pool.tile(SH, F16, tag="fax", name="fax_t")
                fay = io_pool.tile(SH, F16, tag="fay", name="fay_t")
                nc.sync.dma_start(out=fax[:], in_=fax_d[k])
                nc.sync.dma_start(out=fay[:], in_=fay_d[k])

                acc = [d_pool.tile(SH, F32, tag=f"acc{c}", name=f"acc{c}_t")
                       for c in range(3)]
                for c in range(3):
                    nc.scalar.copy(acc[c][:], ps[c][:, 0:W])

                mask = w_pool.tile(SH, F32, tag="uy", name="mask_t")
                mx = w_pool.tile(SH, F32, tag="ux", name="mx_t")
                rec = w_pool.tile(SH, F32, tag="uy", name="rec_t")
                nc.vector.tensor_scalar(mask[:], acc[2][:], THRESH, None,
                                        ALU.is_gt)
                nc.vector.tensor_scalar(mx[:], acc[2][:], THRESH, None,
                                        ALU.max)
                nc.vector.reciprocal(rec[:], mx[:])
                recm = w_pool.tile(SH, F32, tag="ux", name="recm_t")
                nc.vector.tensor_mul(recm[:], rec[:], mask[:])
                for c, (fa, od) in enumerate(((fax, outx_d), (fay, outy_d))):
                    o1 = w_pool.tile(SH, F16, tag="tx", name=f"o1{c}_t")
                    o2 = o_pool.tile(SH, F16, tag=f"o2{c}", name=f"o2{c}_t")
                    nc.vector.tensor_mul(o1[:], acc[c][:], recm[:])
                    nc.vector.tensor_add(o2[:], o1[:], fa[:])
                    nc.sync.dma_start(out=od[k], in_=o2[:])

            if reps == 1:
                for k in range(SLOTS):
                    slot_body(k)
            else:
                with tc.For_i(0, reps, 1):
                    for k in range(SLOTS):
                        slot_body(k)
    nc.finalize()
    return nc


def _prepare(flowAB, back_flowAB, flowBC, imgB_depth):
    """Host marshaling. Returns (active, in_maps)."""
    flowAB = np.asarray(flowAB, dtype=np.float32)
    back = np.asarray(back_flowAB, dtype=np.float32)
    fbc = np.asarray(flowBC, dtype=np.float32)
    dep = np.asarray(imgB_depth, dtype=np.float32)

    xx = np.arange(W, dtype=np.float32)[None, :]
    yy = np.arange(H, dtype=np.float32)[:, None]
    # displacements reproducing reference's fl(x+bx)-x exactly (fp32)
    bx = (xx + back[:, 0]) - xx
    by = (yy + back[:, 1]) - yy

    active = _active_sets(bx, by)

    band = np.zeros((PP, BANDW), np.float16)
    for q in range(PP):
        band[q, q + HALO] = 1.0

    names_pad = {"bx": PADV, "by": PADV}
    dtypes = {"bx": np.float32, "by": np.float32}
    planes = {"fbx": fbc[:, 0], "fby": fbc[:, 1], "dep": dep[:, 0],
              "fax": flowAB[:, 0], "fay": flowAB[:, 1],
              "bx": bx, "by": by}

    in_maps = []
    for core in range(NCORES):
        m = {n: np.zeros((SLOTS, PP, W), dtypes.get(n, np.float16))
             for n in planes}
        m["band"] = band
        for ki in range(IPC):
            b = core * IPC + ki
            for si in range(NS):
                k = ki * NS + si
                y0 = _strip_rows(si)
                lo, hi = max(0, y0), min(H, y0 + PP)
                for n, pl in planes.items():
                    dst = m[n][k]
                    if n in names_pad:
                        dst[:] = names_pad[n]
                    dst[lo - y0:hi - y0, :] = pl[b, lo:hi, :]
        in_maps.append(m)
    return active, in_maps


def kernel(flowAB, back_flowAB, flowBC, imgB_depth):
    active, in_maps = _prepare(flowAB, back_flowAB, flowBC, imgB_depth)
    if active not in _CACHE:
        _CACHE[active] = build_program(active)
    nc = _CACHE[active]
    res = run_bass_kernel_spmd(nc, in_maps, core_ids=list(range(NCORES)))
    out = np.empty((B, 2, H, W), np.float32)
    for core in range(NCORES):
        r = res.results[core]
        for ki in range(IPC):
            b = core * IPC + ki
            for si in range(NS):
                k = ki * NS + si
                y0 = _strip_rows(si)
                dlo = y0 + HALO
                dhi = min(H, dlo + STRIDE)
                n = dhi - dlo
                out[b, 0, dlo:dhi, :] = r["outx"][k][HALO:HALO + n, :]
                out[b, 1, dlo:dhi, :] = r["outy"][k][HALO:HALO + n, :]
    return out


if __name__ == "__main__":
    # quick self-test against the reference
    sys.path.insert(0, '/root/problem')
    import importlib.util
    spec = importlib.util.spec_from_file_location(
        "reference", "/root/problem/reference.py")
    ref = importlib.util.module_from_spec(spec)
    spec.loader.exec_module(ref)
    inputs = {k: np.asarray(v) for k, v in ref.setup_inputs().items()}
    expected = np.asarray(ref.reference(**inputs))
    got = kernel(**inputs)
    err = np.abs(got - expected)
    rel = err.max() / (np.abs(expected).max() + 1e-30)
    print(f"abs max err: {err.max():.3e}  rel: {rel:.3e}")
